# revision 1
# baseline (speedup 1.0000x reference)
"""Multi-head attention (AttnProcessor) Bass kernel for 8 Trainium2 cores.

Problem: hidden_states [2,2048,1280], Wq/Wk/Wv/Wo [1280,1280], bo [1280],
20 heads x head_dim 64.  out = softmax(q k^T / 8) v @ Wo + bo.

Sharding: 40 (batch, head) units -> 8 cores x 5 heads.  Cores 0-3 take
batch 0, cores 4-7 batch 1; each core gets a 5-head column slice of
Wq/Wk/Wv and the matching row slice of Wo, computes its partial output
projection [S, D], and the host sums the 4 partials per batch and adds bo.

Per-core layout:
  - hsT [D, S] (host-transposed) so qT/kT come out as [hd, S] and v as [S, hd]
  - QKV projections in float32r (full-rate fp32, host pre-rounded inputs)
  - attention + output projection in float16 (separate LDWEIGHTS path keeps
    the PE dense enough to hold the HAM clock at 2.4 GHz)
  - scores computed transposed: S^T[kj, qi] = kT-slice x qT  (K = hd = 64),
    software-pipelined one kj-group ahead of PV so PE never waits on exp
  - exp on ScalarE reads PSUM in [128, 1024] groups
  - PV: lhsT = V augmented with a ones column -> psum rows 0..63 = out^T,
    row 64 = softmax denominator, from the same accumulation chain
  - denominator reciprocal broadcast across partitions via a K=1 matmul;
    the broadcast + normalize of head h are deferred into head h+1's QK
    window (and across chunk boundaries) to keep PE busy
  - each chunk's output projection is emitted inside the next chunk's
    first head so the chunk-boundary normalization tail overlaps matmuls
"""

import os
import sys

for _p in ("/opt/trn_rl_repo",):
    if _p not in sys.path and os.path.isdir(_p):
        sys.path.append(_p)

import numpy as np

import concourse.bass as bass
from concourse import bacc
import concourse.mybir as mybir
import concourse.tile as tile
from concourse.bass_utils import run_bass_kernel_spmd

F32 = mybir.dt.float32
F32R = mybir.dt.float32r
F16 = mybir.dt.float16

B, S, D = 2, 2048, 1280
HEADS = 20
HD = D // HEADS          # 64
N_CORES = 8
NH = (B * HEADS) // N_CORES  # heads per core = 5
P = 128


def r(ap):
    """View an fp32 AP as float32r for full-rate matmul."""
    return ap.bitcast(F32R)


def round_fp32r(x):
    """Round fp32 to the fp32r grid (11-bit mantissa, RNE) on the host."""
    u = np.ascontiguousarray(x, dtype=np.float32).view(np.uint32)
    lsb = (u >> 12) & 1
    u2 = (u + 0x7FF + lsb) & np.uint32(0xFFFFF000)
    return u2.view(np.float32)


def build_nc(s=S, d=D, nh=NH, hd=HD, cw=512):
    """Build the SPMD per-core program.

    s: sequence length, d: model dim, nh: heads on this core, hd: head dim,
    cw: qi chunk width (free-dim of score matmuls).
    """
    assert d % P == 0 and s % P == 0 and s % cw == 0 and cw % P == 0
    kt = d // P              # contraction tiles for projections
    c = nh * hd              # projection width
    n_cw = s // cw           # qi chunks
    n_kj = s // P            # key tiles
    st = s // P              # S tiles of 128
    sm_scale = 1.0 / float(np.sqrt(hd))

    nc = bacc.Bacc("TRN2", target_bir_lowering=False)
    hsT = nc.declare_dram_parameter("hsT", [d, s], F32R, isOutput=False)
    wq = nc.declare_dram_parameter("wq", [d, c], F32R, isOutput=False)
    wk = nc.declare_dram_parameter("wk", [d, c], F32R, isOutput=False)
    wv = nc.declare_dram_parameter("wv", [d, c], F32R, isOutput=False)
    wo = nc.declare_dram_parameter("wo", [c, d], F32R, isOutput=False)
    y = nc.declare_dram_parameter("y", [s, d], F32, isOutput=True)

    hsT_t = hsT[:].rearrange("(ko p) s -> p ko s", p=P)   # [128, kt, s]
    wq_t = wq[:].rearrange("(ko p) c -> p ko c", p=P)
    wk_t = wk[:].rearrange("(ko p) c -> p ko c", p=P)
    wv_t = wv[:].rearrange("(ko p) c -> p ko c", p=P)

    # projection output column chunks (M <= 128)
    mchunks = [(i, min(i + P, c)) for i in range(0, c, P)]

    with tile.TileContext(nc) as tc:
        with tc.tile_pool(name="persist", bufs=1) as persist:
            # ---- persistent SBUF tensors ----
            # qT/kT packed two heads per 128-partition tile
            n_qk_tiles = (c + P - 1) // P
            qT_tiles = [
                persist.tile([min(P, c - i * P), s], F16, name=f"qT{i}")
                for i in range(n_qk_tiles)
            ]
            kT_tiles = [
                persist.tile([min(P, c - i * P), s], F16, name=f"kT{i}")
                for i in range(n_qk_tiles)
            ]
            # v with ones column per head: [128, st, nh, hd+1]
            v_aug = persist.tile([P, st, nh, hd + 1], F16, name="v_aug")
            ones_f32 = persist.tile([P, 1], F32, name="ones_f32")
            nc.vector.memset(ones_f32[:], 1.0)
            ones_r = persist.tile([1, hd], F16, name="ones_r")
            nc.vector.tensor_copy(ones_r[:], ones_f32[0:1, 0:1].to_broadcast((1, hd)))
            # wo rows, per-head chunks [hd, d]
            wo_r = persist.tile([hd, nh, d], F32R, name="wo_r")
            nc.sync.dma_start(
                out=wo_r[:],
                in_=wo[:].rearrange("(h p) d -> p h d", p=hd),
            )
            wo_sb = persist.tile([hd, nh, d], F16, name="wo_sb")
            with nc.allow_low_precision(reason="f16 out projection"):
                nc.vector.tensor_copy(wo_sb[:], wo_r[:].bitcast(F32))

            # ---- phase 1: projections ----
            with (
                tc.tile_pool(name="weights", bufs=1) as wpool,
                tc.tile_pool(name="hstream", bufs=2) as hstream,
                tc.tile_pool(name="ps_proj", bufs=4, space="PSUM") as ps_proj,
            ):
                wq_sb = wpool.tile([P, kt, c], F32R, name="wq_sb")
                wk_sb = wpool.tile([P, kt, c], F32R, name="wk_sb")
                wv_sb = wpool.tile([P, kt, c], F32R, name="wv_sb")
                for k in range(kt):
                    nc.sync.dma_start(out=wq_sb[:, k, :], in_=wq_t[:, k, :])
                for k in range(kt):
                    nc.sync.dma_start(out=wk_sb[:, k, :], in_=wk_t[:, k, :])
                for k in range(kt):
                    nc.sync.dma_start(out=wv_sb[:, k, :], in_=wv_t[:, k, :])

                for ncw in range(n_cw):
                    hs_nc = hstream.tile([P, kt, cw], F32R, name="hs_nc")
                    for k in range(kt):
                        nc.sync.dma_start(
                            out=hs_nc[:, k, :],
                            in_=hsT_t[:, k, ncw * cw : (ncw + 1) * cw],
                        )
                    # qT / kT chunks
                    for w_sb, dst_tiles in ((wq_sb, qT_tiles), (wk_sb, kT_tiles)):
                        for mi, (c0, c1) in enumerate(mchunks):
                            m = c1 - c0
                            ps_q = ps_proj.tile([m, cw], F32, tag="ps_q")
                            for k in range(kt):
                                for nn in range(0, cw, 512):
                                    ne = min(nn + 512, cw)
                                    nc.tensor.matmul(
                                        ps_q[:, nn:ne],
                                        r(w_sb[:, k, c0:c1]),
                                        r(hs_nc[:, k, nn:ne]),
                                        start=(k == 0),
                                        stop=(k == kt - 1),
                                    )
                            with nc.allow_low_precision(reason="fp32r qkT"):
                                nc.vector.tensor_copy(
                                    dst_tiles[mi][:, ncw * cw : (ncw + 1) * cw],
                                    ps_q[:],
                                )
                    # v for the S-tiles inside this chunk
                    for ss in range(cw // P):
                        s_global = ncw * (cw // P) + ss
                        ps_v = ps_proj.tile([P, c], F32, tag="ps_v")
                        for k in range(kt):
                            nc.tensor.matmul(
                                ps_v[:],
                                r(hs_nc[:, k, ss * P : (ss + 1) * P]),
                                r(wv_sb[:, k, :]),
                                start=(k == 0),
                                stop=(k == kt - 1),
                            )
                        nc.any.tensor_copy(
                            v_aug[:, s_global, :, 0:hd],
                            ps_v[:].rearrange("p (h e) -> p h e", h=nh),
                        )
                        nc.any.tensor_copy(
                            v_aug[:, s_global, :, hd : hd + 1],
                            ones_f32[:].to_broadcast((P, nh, 1)),
                        )

            # ---- phases 2+3: attention + output projection ----
            with (
                tc.tile_pool(name="ps_s", bufs=2, space="PSUM") as ps_s_pool,
                tc.tile_pool(name="ps_o", bufs=1, space="PSUM") as ps_o_pool,
                tc.tile_pool(name="ps_y", bufs=2, space="PSUM") as ps_y_pool,
                tc.tile_pool(name="ps_bc", bufs=1, space="PSUM") as ps_bc_pool,
                tc.tile_pool(name="exps", bufs=3) as exps_pool,
                tc.tile_pool(name="small", bufs=4) as small_pool,
                tc.tile_pool(name="otile", bufs=2) as otile_pool,
                tc.tile_pool(name="ystage", bufs=2) as ystage_pool,
            ):
                def emit_qk(ps_s, kT_h, qT_h, g, gw, cw):
                    for sl in range(gw):
                        kj = 2 * g + sl
                        for nn in range(0, cw, 512):
                            ne = min(nn + 512, cw)
                            nc.tensor.matmul(
                                ps_s[:, sl * cw + nn : sl * cw + ne],
                                kT_h[:, kj * P : (kj + 1) * P],
                                qT_h[:, nn:ne],
                                start=True,
                                stop=True,
                            )

                def emit_pv(ps_o, expS, h, g, gw, cw):
                    for sl in range(gw):
                        kj = 2 * g + sl
                        for nn in range(0, cw, 512):
                            ne = min(nn + 512, cw)
                            nc.tensor.matmul(
                                ps_o[:, nn:ne],
                                v_aug[:, kj, h, :],
                                expS[:, sl * cw + nn : sl * cw + ne],
                                start=(kj == 0),
                                stop=(kj == n_kj - 1),
                            )

                n_groups = max(1, n_kj // 2)

                def emit_flush(pend):
                    """Deferred normalization: broadcast denom, multiply."""
                    prs, po_un, ph, tgt = pend
                    ps_bc = ps_bc_pool.tile([hd, cw], F32, tag="ps_bc",
                                            name="ps_bc")
                    for nn in range(0, cw, 512):
                        ne = min(nn + 512, cw)
                        nc.tensor.matmul(
                            ps_bc[:, nn:ne],
                            ones_r[:],
                            prs[:, nn:ne],
                            start=True,
                            stop=True,
                        )
                    oT = otile_pool.tile([hd, cw], F16, tag=f"oT{ph}",
                                         name="oT")
                    with nc.allow_low_precision(reason="f16 attn out"):
                        nc.vector.tensor_mul(oT[:], po_un[:], ps_bc[:])
                    tgt.append(oT)
                    return ph

                def emit_proj(outT_ch, pncw):
                    for tt in range(cw // P):
                        t_lo = (pncw * (cw // P) + tt) * P
                        tl = tt * P
                        y_sb = ystage_pool.tile([P, d], F32, tag="y_sb",
                                                name="y_sb")
                        for nn in range(0, d, 512):
                            ne = min(nn + 512, d)
                            ps_y = ps_y_pool.tile([P, 512], F32, tag="ps_y",
                                                  name="ps_y")
                            for h in range(nh):
                                nc.tensor.matmul(
                                    ps_y[:, : ne - nn],
                                    outT_ch[h][:, tl : tl + P],
                                    wo_sb[:, h, nn:ne],
                                    start=(h == 0),
                                    stop=(h == nh - 1),
                                )
                            nc.vector.tensor_copy(
                                y_sb[:, nn:ne], ps_y[:, : ne - nn]
                            )
                        nc.sync.dma_start(
                            out=y[t_lo : t_lo + P, :], in_=y_sb[:]
                        )

                pending = None        # (rs, o_un, h, target outT list)
                proj_wait = None      # (outT list, ncw) ready once filled
                for ncw in range(n_cw):
                    q_lo, q_hi = ncw * cw, (ncw + 1) * cw
                    outT_ch = []
                    for h in range(nh):
                        ht, hoff = h // 2, (h % 2) * hd
                        qT_h = qT_tiles[ht][hoff : hoff + hd, q_lo:q_hi]
                        kT_h = kT_tiles[ht][hoff : hoff + hd, :]
                        ps_o = ps_o_pool.tile([hd + 1, cw], F32, tag="ps_o",
                                              name="ps_o")
                        ps_s_list = [None] * n_groups
                        gw0 = min(2, n_kj)
                        ps_s_list[0] = ps_s_pool.tile(
                            [P, gw0 * cw], F32, tag="ps_s", name="ps_s"
                        )
                        emit_qk(ps_s_list[0], kT_h, qT_h, 0, gw0, cw)
                        # previous head's deferred normalization runs while
                        # this head's QK keeps PE busy; completing the last
                        # head of a chunk releases that chunk's projection
                        if pending is not None:
                            done_h = emit_flush(pending)
                            pending = None
                            if done_h == nh - 1 and proj_wait is not None:
                                emit_proj(*proj_wait)
                                proj_wait = None
                        for g in range(n_groups):
                            if g + 1 < n_groups:
                                gw1 = min(2, n_kj - 2 * (g + 1))
                                ps_s_list[g + 1] = ps_s_pool.tile(
                                    [P, gw1 * cw], F32, tag="ps_s", name="ps_s"
                                )
                                emit_qk(ps_s_list[g + 1], kT_h, qT_h,
                                        g + 1, gw1, cw)
                            gw = min(2, n_kj - 2 * g)
                            expS = exps_pool.tile([P, gw * cw], F16,
                                                  tag="expS", name="expS")
                            nc.scalar.activation(
                                expS[:], ps_s_list[g][:],
                                mybir.ActivationFunctionType.Exp,
                                scale=sm_scale,
                            )
                            emit_pv(ps_o, expS, h, g, gw, cw)
                        rs = small_pool.tile([1, cw], F16, tag="rs", name="rs")
                        with nc.allow_low_precision(reason="fp32r denom"):
                            nc.vector.tensor_copy(rs[:], ps_o[hd : hd + 1, :])
                            nc.vector.reciprocal(rs[:], rs[:])
                        o_un = small_pool.tile([hd, cw], F32, tag="o_un",
                                               name="o_un")
                        nc.vector.tensor_copy(o_un[:], ps_o[0:hd, :])
                        pending = (rs, o_un, h, outT_ch)
                    proj_wait = (outT_ch, ncw)
                # tail: flush last head and project the final chunk
                emit_flush(pending)
                emit_proj(*proj_wait)
    nc.compile()
    return nc


_NC_CACHE = {}


def _get_nc():
    key = (S, D, NH, HD)
    if key not in _NC_CACHE:
        _NC_CACHE[key] = build_nc()
    return _NC_CACHE[key]


def shard_inputs(hidden_states, Wq, Wk, Wv, Wo):
    """Build the 8 per-core input maps."""
    hs = np.asarray(hidden_states, dtype=np.float32)
    hsT = [np.ascontiguousarray(hs[b].T) for b in range(B)]  # [D, S] each
    Wo = np.asarray(Wo, dtype=np.float32)
    in_maps = []
    cores_per_b = N_CORES // B
    for core in range(N_CORES):
        b = core // cores_per_b
        h0 = (core % cores_per_b) * NH
        cols = slice(h0 * HD, (h0 + NH) * HD)
        in_maps.append(
            {
                "hsT": round_fp32r(hsT[b]),
                "wq": round_fp32r(np.asarray(Wq, np.float32)[:, cols]),
                "wk": round_fp32r(np.asarray(Wk, np.float32)[:, cols]),
                "wv": round_fp32r(np.asarray(Wv, np.float32)[:, cols]),
                "wo": round_fp32r(Wo[cols, :]),
            }
        )
    return in_maps


def kernel(hidden_states, Wq, Wk, Wv, Wo, bo, trace=False):
    nc = _get_nc()
    in_maps = shard_inputs(hidden_states, Wq, Wk, Wv, Wo)
    res = run_bass_kernel_spmd(
        nc, in_maps, core_ids=list(range(N_CORES)), trace=trace
    )
    cores_per_b = N_CORES // B
    out = np.empty((B, S, D), dtype=np.float32)
    bo32 = np.asarray(bo, dtype=np.float32)
    for b in range(B):
        acc = res.results[b * cores_per_b]["y"].astype(np.float32)
        for i in range(1, cores_per_b):
            acc = acc + res.results[b * cores_per_b + i]["y"]
        out[b] = acc + bo32
    if trace:
        kernel.last_exec_time_ns = res.exec_time_ns
        kernel.last_results = res
    return out



# revision 7
# speedup vs baseline: 1.1173x; 1.1173x over previous
"""Multi-head attention (AttnProcessor) Bass kernel for 8 Trainium2 cores.

Problem: hidden_states [2,2048,1280], Wq/Wk/Wv/Wo [1280,1280], bo [1280],
20 heads x head_dim 64.  out = softmax(q k^T / 8) v @ Wo + bo.

Sharding: 40 (batch, head) units -> 8 cores x 5 heads.  Cores 0-3 take
batch 0, cores 4-7 batch 1; each core gets a 5-head column slice of
Wq/Wk/Wv and the matching row slice of Wo, computes its partial output
projection [S, D], and the host sums the 4 partials per batch and adds bo.

Per-core layout:
  - hsT [D, S] (host-transposed) so qT/kT come out as [hd, S] and v as [S, hd]
  - QKV projections in float32r (full-rate fp32, host pre-rounded inputs)
  - attention + output projection in float16 (separate LDWEIGHTS path keeps
    the PE dense enough to hold the HAM clock at 2.4 GHz)
  - scores computed transposed: S^T[kj, qi] = kT-slice x qT  (K = hd = 64),
    software-pipelined one kj-group ahead of PV so PE never waits on exp
  - exp on ScalarE reads PSUM in [128, 1024] groups
  - PV: lhsT = V augmented with a ones column -> psum rows 0..63 = out^T,
    row 64 = softmax denominator, from the same accumulation chain
  - normalization runs entirely off the PE: DVE reciprocal_approx_fast on
    the denominator row, GpSimd partition_broadcast across the 64 head-dim
    partitions, DVE multiply straight out of PSUM (ps_o double-buffered).
    The PE instruction stream never waits on a DVE/ScalarE result, so the
    HAM clock stays at 2.4 GHz instead of re-throttling each head.
  - each chunk's output projection is emitted a few QK groups into the
    next chunk's first head so its oT inputs are ready when PE gets there
"""

import os
import sys

for _p in ("/opt/trn_rl_repo",):
    if _p not in sys.path and os.path.isdir(_p):
        sys.path.append(_p)

import numpy as np

import concourse.bass as bass
from concourse import bacc
import concourse.mybir as mybir
import concourse.tile as tile
from concourse.bass_utils import run_bass_kernel_spmd

F32 = mybir.dt.float32
F32R = mybir.dt.float32r
F16 = mybir.dt.float16

B, S, D = 2, 2048, 1280
HEADS = 20
HD = D // HEADS          # 64
N_CORES = 8
NH = (B * HEADS) // N_CORES  # heads per core = 5
P = 128


def r(ap):
    """View an fp32 AP as float32r for full-rate matmul."""
    return ap.bitcast(F32R)


def round_fp32r(x):
    """Round fp32 to the fp32r grid (11-bit mantissa, RNE) on the host."""
    u = np.ascontiguousarray(x, dtype=np.float32).view(np.uint32)
    lsb = (u >> 12) & 1
    u2 = (u + 0x7FF + lsb) & np.uint32(0xFFFFF000)
    return u2.view(np.float32)


def build_nc(s=S, d=D, nh=NH, hd=HD, cw=512):
    """Build the SPMD per-core program.

    s: sequence length, d: model dim, nh: heads on this core, hd: head dim,
    cw: qi chunk width (free-dim of score matmuls).
    """
    assert d % P == 0 and s % P == 0 and s % cw == 0 and cw % P == 0
    kt = d // P              # contraction tiles for projections
    c = nh * hd              # projection width
    n_cw = s // cw           # qi chunks
    n_kj = s // P            # key tiles
    st = s // P              # S tiles of 128
    sm_scale = 1.0 / float(np.sqrt(hd))

    nc = bacc.Bacc("TRN2", target_bir_lowering=False)
    hsT = nc.declare_dram_parameter("hsT", [d, s], F32R, isOutput=False)
    wq = nc.declare_dram_parameter("wq", [d, c], F32R, isOutput=False)
    wk = nc.declare_dram_parameter("wk", [d, c], F32R, isOutput=False)
    wv = nc.declare_dram_parameter("wv", [d, c], F32R, isOutput=False)
    wo = nc.declare_dram_parameter("wo", [c, d], F32R, isOutput=False)
    y = nc.declare_dram_parameter("y", [s, d], F32, isOutput=True)

    hsT_t = hsT[:].rearrange("(ko p) s -> p ko s", p=P)   # [128, kt, s]
    wq_t = wq[:].rearrange("(ko p) c -> p ko c", p=P)
    wk_t = wk[:].rearrange("(ko p) c -> p ko c", p=P)
    wv_t = wv[:].rearrange("(ko p) c -> p ko c", p=P)

    # projection output column chunks (M <= 128)
    mchunks = [(i, min(i + P, c)) for i in range(0, c, P)]

    with tile.TileContext(nc) as tc:
        with tc.tile_pool(name="persist", bufs=1) as persist:
            # ---- persistent SBUF tensors ----
            # qT/kT packed two heads per 128-partition tile
            n_qk_tiles = (c + P - 1) // P
            qT_tiles = [
                persist.tile([min(P, c - i * P), s], F16, name=f"qT{i}")
                for i in range(n_qk_tiles)
            ]
            kT_tiles = [
                persist.tile([min(P, c - i * P), s], F16, name=f"kT{i}")
                for i in range(n_qk_tiles)
            ]
            # v with ones column per head: [128, st, nh, hd+1]
            v_aug = persist.tile([P, st, nh, hd + 1], F16, name="v_aug")
            ones_f32 = persist.tile([P, 1], F32, name="ones_f32")
            nc.vector.memset(ones_f32[:], 1.0)
            # wo rows, per-head chunks [hd, d]
            wo_r = persist.tile([hd, nh, d], F32R, name="wo_r")
            nc.sync.dma_start(
                out=wo_r[:],
                in_=wo[:].rearrange("(h p) d -> p h d", p=hd),
            )
            wo_sb = persist.tile([hd, nh, d], F16, name="wo_sb")
            with nc.allow_low_precision(reason="f16 out projection"):
                nc.vector.tensor_copy(wo_sb[:], wo_r[:].bitcast(F32))

            # ---- phase 1: projections ----
            with (
                tc.tile_pool(name="weights", bufs=1) as wpool,
                tc.tile_pool(name="hstream", bufs=2) as hstream,
                tc.tile_pool(name="ps_proj", bufs=4, space="PSUM") as ps_proj,
            ):
                wq_sb = wpool.tile([P, kt, c], F32R, name="wq_sb")
                wk_sb = wpool.tile([P, kt, c], F32R, name="wk_sb")
                wv_sb = wpool.tile([P, kt, c], F32R, name="wv_sb")
                for k in range(kt):
                    nc.sync.dma_start(out=wq_sb[:, k, :], in_=wq_t[:, k, :])
                for k in range(kt):
                    nc.sync.dma_start(out=wk_sb[:, k, :], in_=wk_t[:, k, :])
                for k in range(kt):
                    nc.sync.dma_start(out=wv_sb[:, k, :], in_=wv_t[:, k, :])

                for ncw in range(n_cw):
                    hs_nc = hstream.tile([P, kt, cw], F32R, name="hs_nc")
                    for k in range(kt):
                        nc.sync.dma_start(
                            out=hs_nc[:, k, :],
                            in_=hsT_t[:, k, ncw * cw : (ncw + 1) * cw],
                        )
                    # qT / kT chunks
                    for w_sb, dst_tiles in ((wq_sb, qT_tiles), (wk_sb, kT_tiles)):
                        for mi, (c0, c1) in enumerate(mchunks):
                            m = c1 - c0
                            ps_q = ps_proj.tile([m, cw], F32, tag="ps_q")
                            for k in range(kt):
                                for nn in range(0, cw, 512):
                                    ne = min(nn + 512, cw)
                                    nc.tensor.matmul(
                                        ps_q[:, nn:ne],
                                        r(w_sb[:, k, c0:c1]),
                                        r(hs_nc[:, k, nn:ne]),
                                        start=(k == 0),
                                        stop=(k == kt - 1),
                                    )
                            with nc.allow_low_precision(reason="fp32r qkT"):
                                nc.vector.tensor_copy(
                                    dst_tiles[mi][:, ncw * cw : (ncw + 1) * cw],
                                    ps_q[:],
                                )
                    # v for the S-tiles inside this chunk
                    for ss in range(cw // P):
                        s_global = ncw * (cw // P) + ss
                        ps_v = ps_proj.tile([P, c], F32, tag="ps_v")
                        for k in range(kt):
                            nc.tensor.matmul(
                                ps_v[:],
                                r(hs_nc[:, k, ss * P : (ss + 1) * P]),
                                r(wv_sb[:, k, :]),
                                start=(k == 0),
                                stop=(k == kt - 1),
                            )
                        nc.any.tensor_copy(
                            v_aug[:, s_global, :, 0:hd],
                            ps_v[:].rearrange("p (h e) -> p h e", h=nh),
                        )
                        nc.any.tensor_copy(
                            v_aug[:, s_global, :, hd : hd + 1],
                            ones_f32[:].to_broadcast((P, nh, 1)),
                        )

            # ---- phases 2+3: attention + output projection ----
            with (
                tc.tile_pool(name="ps_s", bufs=2, space="PSUM") as ps_s_pool,
                tc.tile_pool(name="ps_o", bufs=2, space="PSUM") as ps_o_pool,
                tc.tile_pool(name="ps_y", bufs=2, space="PSUM") as ps_y_pool,
                tc.tile_pool(name="exps", bufs=3) as exps_pool,
                tc.tile_pool(name="small", bufs=4) as small_pool,
                tc.tile_pool(name="otile", bufs=2) as otile_pool,
                tc.tile_pool(name="ystage", bufs=2) as ystage_pool,
            ):
                def emit_qk(ps_s, kT_h, qT_h, g, gw, cw):
                    for sl in range(gw):
                        kj = 2 * g + sl
                        for nn in range(0, cw, 512):
                            ne = min(nn + 512, cw)
                            nc.tensor.matmul(
                                ps_s[:, sl * cw + nn : sl * cw + ne],
                                kT_h[:, kj * P : (kj + 1) * P],
                                qT_h[:, nn:ne],
                                start=True,
                                stop=True,
                            )

                def emit_pv(ps_o, expS, h, g, gw, cw):
                    for sl in range(gw):
                        kj = 2 * g + sl
                        for nn in range(0, cw, 512):
                            ne = min(nn + 512, cw)
                            nc.tensor.matmul(
                                ps_o[:, nn:ne],
                                v_aug[:, kj, h, :],
                                expS[:, sl * cw + nn : sl * cw + ne],
                                start=(kj == 0),
                                stop=(kj == n_kj - 1),
                            )

                n_groups = max(1, n_kj // 2)

                def emit_flush(ps_o, h, tgt):
                    """Normalization, entirely off the PE queue: DVE
                    approx-reciprocal of the denominator row, GpSimd
                    partition-broadcast, DVE multiply from PSUM."""
                    # custom DVE ops read partition 0 regardless of the AP's
                    # base partition -- stage the denominator row into SBUF
                    den = small_pool.tile([1, cw], F32, tag="den", name="den")
                    nc.vector.tensor_copy(den[:], ps_o[hd : hd + 1, :])
                    rcp = small_pool.tile([1, cw], F32, tag="rcp", name="rcp")
                    nc.vector.reciprocal_approx_fast(rcp[:], den[:])
                    rcp_bc = small_pool.tile([hd, cw], F32, tag="rcp_bc",
                                             name="rcp_bc")
                    nc.gpsimd.partition_broadcast(rcp_bc[:], rcp[:])
                    oT = otile_pool.tile([hd, cw], F16, tag=f"oT{h}",
                                         name="oT")
                    with nc.allow_low_precision(reason="f16 attn out"):
                        nc.vector.tensor_mul(oT[:], ps_o[0:hd, :], rcp_bc[:])
                    tgt.append(oT)

                def emit_proj(outT_ch, pncw):
                    for tt in range(cw // P):
                        t_lo = (pncw * (cw // P) + tt) * P
                        tl = tt * P
                        y_sb = ystage_pool.tile([P, d], F32, tag="y_sb",
                                                name="y_sb")
                        for nn in range(0, d, 512):
                            ne = min(nn + 512, d)
                            ps_y = ps_y_pool.tile([P, 512], F32, tag="ps_y",
                                                  name="ps_y")
                            for h in range(nh):
                                nc.tensor.matmul(
                                    ps_y[:, : ne - nn],
                                    outT_ch[h][:, tl : tl + P],
                                    wo_sb[:, h, nn:ne],
                                    start=(h == 0),
                                    stop=(h == nh - 1),
                                )
                            nc.vector.tensor_copy(
                                y_sb[:, nn:ne], ps_y[:, : ne - nn]
                            )
                        nc.sync.dma_start(
                            out=y[t_lo : t_lo + P, :], in_=y_sb[:]
                        )

                proj_wait = None      # (outT list, ncw) ready once filled
                for ncw in range(n_cw):
                    q_lo, q_hi = ncw * cw, (ncw + 1) * cw
                    outT_ch = []
                    for h in range(nh):
                        ht, hoff = h // 2, (h % 2) * hd
                        qT_h = qT_tiles[ht][hoff : hoff + hd, q_lo:q_hi]
                        kT_h = kT_tiles[ht][hoff : hoff + hd, :]
                        ps_o = ps_o_pool.tile([hd + 1, cw], F32, tag="ps_o",
                                              name="ps_o")
                        ps_s_list = [None] * n_groups
                        gw0 = min(2, n_kj)
                        ps_s_list[0] = ps_s_pool.tile(
                            [P, gw0 * cw], F32, tag="ps_s", name="ps_s"
                        )
                        emit_qk(ps_s_list[0], kT_h, qT_h, 0, gw0, cw)
                        for g in range(n_groups):
                            if g + 1 < n_groups:
                                gw1 = min(2, n_kj - 2 * (g + 1))
                                ps_s_list[g + 1] = ps_s_pool.tile(
                                    [P, gw1 * cw], F32, tag="ps_s", name="ps_s"
                                )
                                emit_qk(ps_s_list[g + 1], kT_h, qT_h,
                                        g + 1, gw1, cw)
                            gw = min(2, n_kj - 2 * g)
                            expS = exps_pool.tile([P, gw * cw], F16,
                                                  tag="expS", name="expS")
                            nc.scalar.activation(
                                expS[:], ps_s_list[g][:],
                                mybir.ActivationFunctionType.Exp,
                                scale=sm_scale,
                            )
                            emit_pv(ps_o, expS, h, g, gw, cw)
                            # the previous chunk's output projection goes
                            # out a few groups into this chunk's first head
                            # so its oT inputs (last head's DVE normalize)
                            # are done before PE reaches these matmuls
                            if h == 0 and g == 2 and proj_wait is not None:
                                emit_proj(*proj_wait)
                                proj_wait = None
                        emit_flush(ps_o, h, outT_ch)
                    proj_wait = (outT_ch, ncw)
                # tail: project the final chunk
                emit_proj(*proj_wait)
    nc.compile()
    return nc


_NC_CACHE = {}


def _get_nc():
    key = (S, D, NH, HD)
    if key not in _NC_CACHE:
        _NC_CACHE[key] = build_nc()
    return _NC_CACHE[key]


def shard_inputs(hidden_states, Wq, Wk, Wv, Wo):
    """Build the 8 per-core input maps."""
    hs = np.asarray(hidden_states, dtype=np.float32)
    hsT = [np.ascontiguousarray(hs[b].T) for b in range(B)]  # [D, S] each
    Wo = np.asarray(Wo, dtype=np.float32)
    in_maps = []
    cores_per_b = N_CORES // B
    for core in range(N_CORES):
        b = core // cores_per_b
        h0 = (core % cores_per_b) * NH
        cols = slice(h0 * HD, (h0 + NH) * HD)
        in_maps.append(
            {
                "hsT": round_fp32r(hsT[b]),
                "wq": round_fp32r(np.asarray(Wq, np.float32)[:, cols]),
                "wk": round_fp32r(np.asarray(Wk, np.float32)[:, cols]),
                "wv": round_fp32r(np.asarray(Wv, np.float32)[:, cols]),
                "wo": round_fp32r(Wo[cols, :]),
            }
        )
    return in_maps


def kernel(hidden_states, Wq, Wk, Wv, Wo, bo, trace=False):
    nc = _get_nc()
    in_maps = shard_inputs(hidden_states, Wq, Wk, Wv, Wo)
    res = run_bass_kernel_spmd(
        nc, in_maps, core_ids=list(range(N_CORES)), trace=trace
    )
    cores_per_b = N_CORES // B
    out = np.empty((B, S, D), dtype=np.float32)
    bo32 = np.asarray(bo, dtype=np.float32)
    for b in range(B):
        acc = res.results[b * cores_per_b]["y"].astype(np.float32)
        for i in range(1, cores_per_b):
            acc = acc + res.results[b * cores_per_b + i]["y"]
        out[b] = acc + bo32
    if trace:
        kernel.last_exec_time_ns = res.exec_time_ns
        kernel.last_results = res
    return out



# revision 9
# speedup vs baseline: 1.4186x; 1.2697x over previous
"""Multi-head attention (AttnProcessor) Bass kernel for 8 Trainium2 cores.

Problem: hidden_states [2,2048,1280], Wq/Wk/Wv/Wo [1280,1280], bo [1280],
20 heads x head_dim 64.  out = softmax(q k^T / 8) v @ Wo + bo.

Sharding: 40 (batch, head) units -> 8 cores x 5 heads.  Cores 0-3 take
batch 0, cores 4-7 batch 1; each core gets a 5-head column slice of
Wq/Wk/Wv and the matching row slice of Wo, computes its partial output
projection [S, D], and the host sums the 4 partials per batch and adds bo.

Per-core schedule (the HAM clock gate drops the PE to 1.2 GHz whenever the
PE idles, so the whole design keeps the PE instruction stream gap-free):

  phase 1 (dense matmuls, no cross-engine deps):
    kT for the full sequence, v for the full sequence, qT for qi chunk 0.
  phase 2 (attention, one (head, group) unit at a time):
    per unit: QK of the NEXT unit -> 2-3 filler matmuls -> PV of this
    unit.  The filler stream carries the qT projections for later qi
    chunks and the output projection of the previous chunk, sized so the
    PE always has more queued work than the ScalarE exp latency.
  scores are computed transposed (S^T = kT-slice x qT, K=hd=64) so PV
    needs no transpose and the ones-augmented V gives the softmax
    denominator in the same PSUM accumulation chain.
  normalization runs entirely off the PE: DVE copy of the denominator row,
    DVE reciprocal_approx_fast, GpSimd partition_broadcast, DVE multiply
    straight out of PSUM (ps_o double-buffered).
"""

import os
import sys

for _p in ("/opt/trn_rl_repo",):
    if _p not in sys.path and os.path.isdir(_p):
        sys.path.append(_p)

import numpy as np

import concourse.bass as bass
from concourse import bacc
import concourse.mybir as mybir
import concourse.tile as tile
from concourse.bass_utils import run_bass_kernel_spmd

F32 = mybir.dt.float32
F32R = mybir.dt.float32r
F16 = mybir.dt.float16

B, S, D = 2, 2048, 1280
HEADS = 20
HD = D // HEADS          # 64
N_CORES = 8
NH = (B * HEADS) // N_CORES  # heads per core = 5
P = 128


def r(ap):
    """View an fp32 AP as float32r for full-rate matmul."""
    return ap.bitcast(F32R)


def round_fp32r(x):
    """Round fp32 to the fp32r grid (11-bit mantissa, RNE) on the host."""
    u = np.ascontiguousarray(x, dtype=np.float32).view(np.uint32)
    lsb = (u >> 12) & 1
    u2 = (u + 0x7FF + lsb) & np.uint32(0xFFFFF000)
    return u2.view(np.float32)


def build_nc(s=S, d=D, nh=NH, hd=HD, cw=512):
    """Build the SPMD per-core program.

    s: sequence length, d: model dim, nh: heads on this core, hd: head dim,
    cw: qi chunk width (free-dim of score matmuls).
    """
    assert d % P == 0 and s % P == 0 and s % cw == 0 and cw % P == 0
    kt = d // P              # contraction tiles for projections
    c = nh * hd              # projection width
    n_cw = s // cw           # qi chunks
    n_kj = s // P            # key tiles
    st = s // P              # S tiles of 128
    n_groups = max(1, n_kj // 2)
    sm_scale = 1.0 / float(np.sqrt(hd))

    nc = bacc.Bacc("TRN2", target_bir_lowering=False)
    hsT = nc.declare_dram_parameter("hsT", [d, s], F32R, isOutput=False)
    wq = nc.declare_dram_parameter("wq", [d, c], F32R, isOutput=False)
    wk = nc.declare_dram_parameter("wk", [d, c], F32R, isOutput=False)
    wv = nc.declare_dram_parameter("wv", [d, c], F32R, isOutput=False)
    wo = nc.declare_dram_parameter("wo", [c, d], F16, isOutput=False)
    y = nc.declare_dram_parameter("y", [s, d], F32, isOutput=True)

    hsT_t = hsT[:].rearrange("(ko p) s -> p ko s", p=P)   # [128, kt, s]
    wq_t = wq[:].rearrange("(ko p) c -> p ko c", p=P)
    wk_t = wk[:].rearrange("(ko p) c -> p ko c", p=P)
    wv_t = wv[:].rearrange("(ko p) c -> p ko c", p=P)

    # projection output column chunks (M <= 128)
    mchunks = [(i, min(i + P, c)) for i in range(0, c, P)]

    with tile.TileContext(nc) as tc:
        with tc.tile_pool(name="persist", bufs=1) as persist:
            # ---- persistent SBUF tensors ----
            # qT/kT packed two heads per 128-partition tile
            n_qk_tiles = (c + P - 1) // P
            qT_tiles = [
                persist.tile([min(P, c - i * P), s], F16, name=f"qT{i}")
                for i in range(n_qk_tiles)
            ]
            kT_tiles = [
                persist.tile([min(P, c - i * P), s], F16, name=f"kT{i}")
                for i in range(n_qk_tiles)
            ]
            # v with ones column per head: [128, st, nh, hd+1]
            v_aug = persist.tile([P, st, nh, hd + 1], F16, name="v_aug")
            ones_f32 = persist.tile([P, 1], F32, name="ones_f32")
            nc.vector.memset(ones_f32[:], 1.0)
            # wo rows, per-head chunks [hd, d], f16 straight from the host
            wo_sb = persist.tile([hd, nh, d], F16, name="wo_sb")
            nc.sync.dma_start(
                out=wo_sb[:],
                in_=wo[:].rearrange("(h p) d -> p h d", p=hd),
            )
            # projection weights stay resident (qT chunks 1+ are produced
            # as filler during the attention phase)
            wq_sb = persist.tile([P, kt, c], F32R, name="wq_sb")
            wk_sb = persist.tile([P, kt, c], F32R, name="wk_sb")
            wv_sb = persist.tile([P, kt, c], F32R, name="wv_sb")
            for k in range(kt):
                nc.sync.dma_start(out=wk_sb[:, k, :], in_=wk_t[:, k, :])
            for k in range(kt):
                nc.sync.dma_start(out=wv_sb[:, k, :], in_=wv_t[:, k, :])
            for k in range(kt):
                nc.sync.dma_start(out=wq_sb[:, k, :], in_=wq_t[:, k, :])

            # ---- phase 1: kT (all chunks), v (all), qT chunk 0 ----
            with (
                tc.tile_pool(name="hstream", bufs=2) as hstream,
                tc.tile_pool(name="ps_proj", bufs=4, space="PSUM") as ps_proj,
            ):
                for ncw in range(n_cw):
                    hs_nc = hstream.tile([P, kt, cw], F32R, name="hs_nc")
                    for k in range(kt):
                        nc.sync.dma_start(
                            out=hs_nc[:, k, :],
                            in_=hsT_t[:, k, ncw * cw : (ncw + 1) * cw],
                        )
                    # kT chunk (and qT for chunk 0)
                    srcs = [(wk_sb, kT_tiles)]
                    if ncw == 0:
                        srcs.append((wq_sb, qT_tiles))
                    for w_sb, dst_tiles in srcs:
                        for mi, (c0, c1) in enumerate(mchunks):
                            m = c1 - c0
                            ps_q = ps_proj.tile([m, cw], F32, tag="ps_q")
                            for k in range(kt):
                                nc.tensor.matmul(
                                    ps_q[:],
                                    r(w_sb[:, k, c0:c1]),
                                    r(hs_nc[:, k, :]),
                                    start=(k == 0),
                                    stop=(k == kt - 1),
                                )
                            with nc.allow_low_precision(reason="f16 qkT"):
                                nc.vector.tensor_copy(
                                    dst_tiles[mi][:, ncw * cw : (ncw + 1) * cw],
                                    ps_q[:],
                                )
                    # v for the S-tiles inside this chunk
                    for ss in range(cw // P):
                        s_global = ncw * (cw // P) + ss
                        ps_v = ps_proj.tile([P, c], F32, tag="ps_v")
                        for k in range(kt):
                            nc.tensor.matmul(
                                ps_v[:],
                                r(hs_nc[:, k, ss * P : (ss + 1) * P]),
                                r(wv_sb[:, k, :]),
                                start=(k == 0),
                                stop=(k == kt - 1),
                            )
                        nc.any.tensor_copy(
                            v_aug[:, s_global, :, 0:hd],
                            ps_v[:].rearrange("p (h e) -> p h e", h=nh),
                        )
                        nc.any.tensor_copy(
                            v_aug[:, s_global, :, hd : hd + 1],
                            ones_f32[:].to_broadcast((P, nh, 1)),
                        )

            # ---- phase 2: attention with filler weave ----
            with (
                tc.tile_pool(name="ps_s", bufs=2, space="PSUM") as ps_s_pool,
                tc.tile_pool(name="ps_o", bufs=2, space="PSUM") as ps_o_pool,
                tc.tile_pool(name="ps_fill", bufs=2, space="PSUM") as ps_fill_pool,
                tc.tile_pool(name="hstream2", bufs=2) as hstream2,
                tc.tile_pool(name="exps", bufs=3) as exps_pool,
                tc.tile_pool(name="small", bufs=4) as small_pool,
                tc.tile_pool(name="otile", bufs=2) as otile_pool,
                tc.tile_pool(name="ystage", bufs=2) as ystage_pool,
            ):
                def emit_qk(ps_s, ht, hoff, ncw, g):
                    qT_h = qT_tiles[ht][hoff : hoff + hd,
                                        ncw * cw : (ncw + 1) * cw]
                    for sl in range(2):
                        kj = 2 * g + sl
                        nc.tensor.matmul(
                            ps_s[:, sl * cw : (sl + 1) * cw],
                            kT_tiles[ht][hoff : hoff + hd,
                                         kj * P : (kj + 1) * P],
                            qT_h,
                            start=True,
                            stop=True,
                        )

                def emit_pv(ps_o, expS, h, g):
                    for sl in range(2):
                        kj = 2 * g + sl
                        nc.tensor.matmul(
                            ps_o[:],
                            v_aug[:, kj, h, :],
                            expS[:, sl * cw : (sl + 1) * cw],
                            start=(kj == 0),
                            stop=(kj == n_kj - 1),
                        )

                def emit_flush(ps_o, h, tgt):
                    """Normalization, entirely off the PE queue: DVE
                    approx-reciprocal of the denominator row (staged to
                    partition 0 first -- custom DVE ops ignore the AP base
                    partition), GpSimd broadcast, DVE multiply from PSUM."""
                    den = small_pool.tile([1, cw], F32, tag="den", name="den")
                    nc.vector.tensor_copy(den[:], ps_o[hd : hd + 1, :])
                    rcp = small_pool.tile([1, cw], F32, tag="rcp", name="rcp")
                    nc.vector.reciprocal_approx_fast(rcp[:], den[:])
                    rcp_bc = small_pool.tile([hd, cw], F32, tag="rcp_bc",
                                             name="rcp_bc")
                    nc.gpsimd.partition_broadcast(rcp_bc[:], rcp[:])
                    oT = otile_pool.tile([hd, cw], F16, tag=f"oT{h}",
                                         name="oT")
                    with nc.allow_low_precision(reason="f16 attn out"):
                        nc.vector.tensor_mul(oT[:], ps_o[0:hd, :], rcp_bc[:])
                    tgt.append(oT)

                # -- filler: single-matmul pieces fed between QK and PV --
                def qT_proj_pieces(pncw, hs_nc):
                    """Produce qT for chunk pncw: 3 chains of kt matmuls."""
                    for mi, (c0, c1) in enumerate(mchunks):
                        m = c1 - c0
                        ps_q = ps_fill_pool.tile([m, cw], F32, tag="fill",
                                                 name="fill")

                        def mk(k, ps_q=ps_q, c0=c0, c1=c1, mi=mi, last=None):
                            def go():
                                nc.tensor.matmul(
                                    ps_q[:],
                                    r(wq_sb[:, k, c0:c1]),
                                    r(hs_nc[:, k, :]),
                                    start=(k == 0),
                                    stop=(k == kt - 1),
                                )
                                if k == kt - 1:
                                    with nc.allow_low_precision(reason="f16 q"):
                                        nc.vector.tensor_copy(
                                            qT_tiles[mi][
                                                :, pncw * cw : (pncw + 1) * cw
                                            ],
                                            ps_q[:],
                                        )
                            return go

                        for k in range(kt):
                            yield mk(k)

                def out_proj_pieces(outT_ch, pncw):
                    """Output projection of chunk pncw: per seq-tile, per
                    d-chunk, a chain of nh matmuls then a DVE drain."""
                    for tt in range(cw // P):
                        t_lo = (pncw * (cw // P) + tt) * P
                        tl = tt * P
                        y_sb = ystage_pool.tile([P, d], F32, tag="y_sb",
                                                name="y_sb")
                        for nn in range(0, d, 512):
                            ne = min(nn + 512, d)
                            ps_y = ps_fill_pool.tile([P, ne - nn], F32,
                                                     tag="fill", name="fill")

                            def mk(h, nn=nn, ne=ne, ps_y=ps_y, y_sb=y_sb,
                                   tl=tl, t_lo=t_lo):
                                def go():
                                    nc.tensor.matmul(
                                        ps_y[:],
                                        outT_ch[h][:, tl : tl + P],
                                        wo_sb[:, h, nn:ne],
                                        start=(h == 0),
                                        stop=(h == nh - 1),
                                    )
                                    if h == nh - 1:
                                        nc.vector.tensor_copy(
                                            y_sb[:, nn:ne], ps_y[:]
                                        )
                                        if ne == d:
                                            nc.sync.dma_start(
                                                out=y[t_lo : t_lo + P, :],
                                                in_=y_sb[:],
                                            )
                                return go

                            for h in range(nh):
                                yield mk(h)

                # stream of (ncw, h, g) units with one-unit QK lookahead
                stream = [
                    (ncw, h, g)
                    for ncw in range(n_cw)
                    for h in range(nh)
                    for g in range(n_groups)
                ]
                upg = nh * n_groups  # units per chunk

                # per-chunk filler iterators (built lazily at chunk entry)
                fillers = [None] * n_cw
                outT_by_chunk = [[] for _ in range(n_cw)]

                # stage the hidden-state slices for the filler qT
                # projections up front; the DMAs execute while phase 1 /
                # early attention units run, so no filler piece stalls on
                # them (bufs=2 round-robin ordering is semaphore-tracked)
                hs2_tiles = {}
                for pncw in range(1, n_cw):
                    hs2 = hstream2.tile([P, kt, cw], F32R, name="hs2")
                    for k in range(kt):
                        nc.sync.dma_start(
                            out=hs2[:, k, :],
                            in_=hsT_t[:, k, pncw * cw : (pncw + 1) * cw],
                        )
                    hs2_tiles[pncw] = hs2

                def chunk_filler(ncw):
                    """Filler supply while attention chunk ncw runs."""
                    def chain():
                        if ncw == 0:
                            for pncw in (1, 2):
                                yield from qT_proj_pieces(
                                    pncw, hs2_tiles[pncw]
                                )
                        else:
                            if ncw == 1:
                                yield from qT_proj_pieces(3, hs2_tiles[3])
                            # output projection of the previous chunk
                            yield from out_proj_pieces(
                                outT_by_chunk[ncw - 1], ncw - 1
                            )
                    return chain()

                prev = None  # (ps_s, ncw, h, g, ps_o)
                ps_o = None
                for i, (ncw, h, g) in enumerate(stream):
                    if g == 0 and h == 0:
                        fillers[ncw] = chunk_filler(ncw)
                    ht, hoff = h // 2, (h % 2) * hd
                    ps_s = ps_s_pool.tile([P, 2 * cw], F32, tag="ps_s",
                                          name="ps_s")
                    emit_qk(ps_s, ht, hoff, ncw, g)
                    if prev is not None:
                        p_ps_s, p_ncw, p_h, p_g, p_ps_o = prev
                        # filler between QK and PV keeps the PE ahead of
                        # the ScalarE exp latency
                        fl = fillers[ncw]
                        budget = 3 if (i % 2) else 2
                        for _ in range(budget):
                            piece = next(fl, None)
                            if piece is None:
                                break
                            piece()
                        expS = exps_pool.tile([P, 2 * cw], F16, tag="expS",
                                              name="expS")
                        nc.scalar.activation(
                            expS[:], p_ps_s[:],
                            mybir.ActivationFunctionType.Exp,
                            scale=sm_scale,
                        )
                        emit_pv(p_ps_o, expS, p_h, p_g)
                        if p_g == n_groups - 1:
                            emit_flush(p_ps_o, p_h, outT_by_chunk[p_ncw])
                    if g == 0:
                        ps_o = ps_o_pool.tile([hd + 1, cw], F32, tag="ps_o",
                                              name="ps_o")
                    prev = (ps_s, ncw, h, g, ps_o)
                # tail: last unit's exp+PV+flush, leftover filler, last
                # chunk's output projection
                p_ps_s, p_ncw, p_h, p_g, p_ps_o = prev
                expS = exps_pool.tile([P, 2 * cw], F16, tag="expS",
                                      name="expS")
                nc.scalar.activation(
                    expS[:], p_ps_s[:],
                    mybir.ActivationFunctionType.Exp,
                    scale=sm_scale,
                )
                emit_pv(p_ps_o, expS, p_h, p_g)
                emit_flush(p_ps_o, p_h, outT_by_chunk[p_ncw])
                for fl in fillers:
                    if fl is not None:
                        for piece in fl:
                            piece()
                for piece in out_proj_pieces(outT_by_chunk[n_cw - 1],
                                             n_cw - 1):
                    piece()
    nc.compile()
    return nc


_NC_CACHE = {}


def _get_nc():
    key = (S, D, NH, HD)
    if key not in _NC_CACHE:
        _NC_CACHE[key] = build_nc()
    return _NC_CACHE[key]


def shard_inputs(hidden_states, Wq, Wk, Wv, Wo):
    """Build the 8 per-core input maps."""
    hs = np.asarray(hidden_states, dtype=np.float32)
    hsT = [np.ascontiguousarray(hs[b].T) for b in range(B)]  # [D, S] each
    Wo = np.asarray(Wo, dtype=np.float32)
    in_maps = []
    cores_per_b = N_CORES // B
    for core in range(N_CORES):
        b = core // cores_per_b
        h0 = (core % cores_per_b) * NH
        cols = slice(h0 * HD, (h0 + NH) * HD)
        in_maps.append(
            {
                "hsT": round_fp32r(hsT[b]),
                "wq": round_fp32r(np.asarray(Wq, np.float32)[:, cols]),
                "wk": round_fp32r(np.asarray(Wk, np.float32)[:, cols]),
                "wv": round_fp32r(np.asarray(Wv, np.float32)[:, cols]),
                "wo": np.ascontiguousarray(Wo[cols, :]).astype(np.float16),
            }
        )
    return in_maps


def kernel(hidden_states, Wq, Wk, Wv, Wo, bo, trace=False):
    nc = _get_nc()
    in_maps = shard_inputs(hidden_states, Wq, Wk, Wv, Wo)
    res = run_bass_kernel_spmd(
        nc, in_maps, core_ids=list(range(N_CORES)), trace=trace
    )
    cores_per_b = N_CORES // B
    out = np.empty((B, S, D), dtype=np.float32)
    bo32 = np.asarray(bo, dtype=np.float32)
    for b in range(B):
        acc = res.results[b * cores_per_b]["y"].astype(np.float32)
        for i in range(1, cores_per_b):
            acc = acc + res.results[b * cores_per_b + i]["y"]
        out[b] = acc + bo32
    if trace:
        kernel.last_exec_time_ns = res.exec_time_ns
        kernel.last_results = res
    return out
